# revision 2
# baseline (speedup 1.0000x reference)
"""GatedAttention Trainium2 kernel.

Per-core computation (data-parallel over batch, 1 batch row / core):
  Q = x @ Wq + bq            [N, A]
  K = x @ Wk + bk            [N, A]
  g = sigmoid(Q @ Wv + bv)   [N]
  S = Q @ K^T, diag -> -inf  [N, N]
  P = softmax(S, -1)         (diag prob 0)
  out = (1-g)[:,None] * P + diag(g)

Layout strategy: keep Q^T / K^T in SBUF with A=128 on partitions so each
128-row score tile is a single matmul per 512-col chunk. exp+rowsum fused on
the scalar engine via accum_out; gating and normalization on DVE; 1 MB
output stores.
"""

import numpy as np

import concourse.bass as bass
import concourse.tile as tile
from concourse import bacc, mybir
from concourse.bass_utils import run_bass_kernel_spmd
from concourse.masks import make_identity

B, N, H, A = 8, 2048, 512, 128
NT = N // 128  # 16 row tiles
HC = H // 128  # 4 h chunks
JC = N // 512  # 4 col chunks
F32 = mybir.dt.float32

_CACHE = {}


def build_nc():
    nc = bacc.Bacc(None, target_bir_lowering=False)

    x = nc.dram_tensor("x", [N, H], F32, kind="ExternalInput")
    Wq = nc.dram_tensor("Wq", [H, A], F32, kind="ExternalInput")
    bq = nc.dram_tensor("bq", [A, 1], F32, kind="ExternalInput")
    Wk = nc.dram_tensor("Wk", [H, A], F32, kind="ExternalInput")
    bk = nc.dram_tensor("bk", [A, 1], F32, kind="ExternalInput")
    Wv = nc.dram_tensor("Wv", [A, 1], F32, kind="ExternalInput")
    bv = nc.dram_tensor("bv", [1, 1], F32, kind="ExternalInput")
    out = nc.dram_tensor("out", [N, N], F32, kind="ExternalOutput")

    with tile.TileContext(nc) as tc:
        with (
            tc.tile_pool(name="singles", bufs=1) as singles,
            tc.tile_pool(name="xload", bufs=4) as xload,
            tc.tile_pool(name="ebuf", bufs=3) as ebuf,
            tc.tile_pool(name="small", bufs=8) as small,
            tc.tile_pool(name="psum", bufs=6, space="PSUM") as psum,
            tc.tile_pool(name="psumt", bufs=2, space="PSUM") as psumt,
        ):
            # ---- constants / weights ----
            ident = singles.tile([128, 128], F32)
            make_identity(nc, ident[:])
            neg_ident = singles.tile([128, 128], F32)
            nc.vector.tensor_scalar_mul(neg_ident[:], ident[:], -1.0e9)

            wq_sb = singles.tile([128, HC, A], F32)
            nc.sync.dma_start(wq_sb[:], Wq[:].rearrange("(h p) a -> p h a", p=128))
            wk_sb = singles.tile([128, HC, A], F32)
            nc.sync.dma_start(wk_sb[:], Wk[:].rearrange("(h p) a -> p h a", p=128))
            wv_sb = singles.tile([128, 1], F32)
            nc.sync.dma_start(wv_sb[:], Wv[:])
            bq_sb = singles.tile([128, 1], F32)
            nc.sync.dma_start(bq_sb[:], bq[:])
            bk_sb = singles.tile([128, 1], F32)
            nc.sync.dma_start(bk_sb[:], bk[:])
            bv_sb = singles.tile([128, 1], F32)
            nc.sync.dma_start(
                bv_sb[:],
                bass.AP(tensor=bv.tensor if isinstance(bv, bass.AP) else bv,
                        offset=0, ap=[[0, 128], [1, 1]]),
            )

            # ---- load x and transpose to xt[p_h, hc, n] ----
            xt = singles.tile([128, HC, N], F32)
            for i in range(NT):
                x_tile = xload.tile([128, H], F32)
                nc.sync.dma_start(x_tile[:], x[i * 128:(i + 1) * 128, :])
                for h in range(HC):
                    pt = psumt.tile([128, 128], F32)
                    nc.tensor.transpose(
                        pt[:], x_tile[:, h * 128:(h + 1) * 128], ident[:]
                    )
                    nc.vector.tensor_copy(
                        xt[:, h, i * 128:(i + 1) * 128], pt[:]
                    )

            # ---- Q^T, K^T : [A=128, N] ----
            qt = singles.tile([128, N], F32)
            kt = singles.tile([128, N], F32)
            for j in range(JC):
                sl = slice(j * 512, (j + 1) * 512)
                pq = psum.tile([128, 512], F32, tag="ps")
                for h in range(HC):
                    nc.tensor.matmul(
                        pq[:], wq_sb[:, h, :], xt[:, h, sl],
                        start=(h == 0), stop=(h == HC - 1),
                    )
                nc.vector.tensor_scalar_add(qt[:, sl], pq[:], bq_sb[:])
                pk = psum.tile([128, 512], F32, tag="ps")
                for h in range(HC):
                    nc.tensor.matmul(
                        pk[:], wk_sb[:, h, :], xt[:, h, sl],
                        start=(h == 0), stop=(h == HC - 1),
                    )
                nc.vector.tensor_scalar_add(kt[:, sl], pk[:], bk_sb[:])

            # ---- gate: g = sigmoid(Q @ Wv + bv), laid out [128, NT] ----
            gp = psum.tile([128, NT], F32, tag="ps")
            for i in range(NT):
                nc.tensor.matmul(
                    gp[:, i:i + 1], qt[:, i * 128:(i + 1) * 128], wv_sb[:],
                    start=True, stop=True,
                )
            g_sb = singles.tile([128, NT], F32)
            nc.scalar.activation(
                g_sb[:], gp[:], mybir.ActivationFunctionType.Sigmoid,
                bias=bv_sb[:, 0:1],
            )
            omg = singles.tile([128, NT], F32)  # 1 - g
            nc.scalar.activation(
                omg[:], g_sb[:], mybir.ActivationFunctionType.Copy,
                bias=1.0, scale=-1.0,
            )

            # ---- main loop over 128-row score tiles ----
            for i in range(NT):
                esb = ebuf.tile([128, N], F32)
                partials = small.tile([128, JC], F32)
                jd = i // 4          # col chunk containing the diagonal
                loc = (i % 4) * 128  # offset of diag block inside that chunk
                for j in range(JC):
                    sl = slice(j * 512, (j + 1) * 512)
                    ps = psum.tile([128, 512], F32, tag="ps")
                    nc.tensor.matmul(
                        ps[:], qt[:, i * 128:(i + 1) * 128], kt[:, sl],
                        start=True, stop=True,
                    )
                    if j == jd:
                        nc.vector.tensor_add(
                            ps[:, loc:loc + 128], ps[:, loc:loc + 128],
                            neg_ident[:],
                        )
                    nc.scalar.activation(
                        esb[:, sl], ps[:], mybir.ActivationFunctionType.Exp,
                        accum_out=partials[:, j:j + 1],
                    )
                rs = small.tile([128, 1], F32)
                nc.vector.tensor_reduce(
                    rs[:], partials[:], axis=mybir.AxisListType.X,
                    op=mybir.AluOpType.add,
                )
                rinv = small.tile([128, 1], F32)
                nc.vector.reciprocal(rinv[:], rs[:])
                scale_i = small.tile([128, 1], F32)
                nc.vector.tensor_mul(scale_i[:], rinv[:], omg[:, i:i + 1])
                gid = small.tile([128, 128], F32, tag="gid")
                nc.vector.tensor_scalar_mul(gid[:], ident[:], g_sb[:, i:i + 1])
                nc.vector.tensor_scalar_mul(esb[:], esb[:], scale_i[:])
                nc.vector.tensor_add(
                    esb[:, i * 128:(i + 1) * 128],
                    esb[:, i * 128:(i + 1) * 128], gid[:],
                )
                nc.sync.dma_start(out[i * 128:(i + 1) * 128, :], esb[:])

    nc.compile()
    return nc


def kernel(x, Wq, bq, Wk, bk, Wv, bv, **_unused):
    if "nc" not in _CACHE:
        _CACHE["nc"] = build_nc()
    nc = _CACHE["nc"]

    f = np.ascontiguousarray
    in_maps = []
    for b in range(B):
        in_maps.append({
            "x": f(x[b], dtype=np.float32),
            "Wq": f(Wq, dtype=np.float32),
            "bq": f(bq, dtype=np.float32).reshape(A, 1),
            "Wk": f(Wk, dtype=np.float32),
            "bk": f(bk, dtype=np.float32).reshape(A, 1),
            "Wv": f(Wv, dtype=np.float32).reshape(A, 1),
            "bv": f(bv, dtype=np.float32).reshape(1, 1),
        })
    res = run_bass_kernel_spmd(nc, in_maps, core_ids=list(range(B)))
    return np.stack([r["out"] for r in res.results], axis=0)
